# revision 73
# baseline (speedup 1.0000x reference)
"""Trainium2 Bass kernel for nn_ContrastiveLoss (ragged_sequence).

Math (see reference): a cross-attention t2i score matrix scores[i, c] over
B=64 images x B=64 captions, then a max-violation margin loss.

Sharding (2-way images x 4-way captions over 8 NeuronCores): each core
holds 32 images and 16 captions and computes its [32, 16] block of the
score matrix; the tiny 64x64 margin-loss reduction runs on host.

Ragged packing: captions are sorted by length and dealt round-robin to
(caption-col, slot) so slot j holds the 4 captions of rank 4j..4j+3.
Slots are padded to four uniform group widths (4 slots per group), so
per-caption word reductions are 4 static strided views instead of 16
ragged ones.  NCW = 4*sum(widths) (~496 for the reference inputs vs 800
dense) and is kept in [256, 512]: >= 256 so fp32r matmuls stream at
1 row/cycle, <= 512 so every [*, NCW] f32 PSUM tile fits one 2 KiB bank.

Device layout (per core):
  Images are processed in 11 "packs" of 3 (32 real + 1 zero-pad).  A pack
  occupies 108 partitions = 3 images x 36 regions.  The main matmul
  A = im @ s^T uses stationary = im-pack [128d, 108br] and moving =
  s^T [128d, NCW], both bf16 (PSUM accumulates f32; the downstream
  normalization cancels most of the rounding), 8 K-chunks of d.
  Word-axis (w) reductions are free-axis DVE reduces over the group
  views; region-axis (r) reductions are PE matmuls:
    - H = Gbd @ E with Gbd = blockdiag(G[b0],G[b1],G[b2]) Gram blocks,
      all packs' stationaries column-concatenated in one SBUF tile
    - NZ/WZ accumulate ones_p^T @ {E*A, E*H} over packs in PSUM, where
      ones_p is a 66-column slice of one sliding-window [108, 132] tile.
  The softmax normalizer Z cancels in sim = (NZ/Z)/(cn*sqrt(WZ)/Z), so it
  is never computed.  The per-pack chain is software-pipelined four deep
  (A(p+1) | Prelu(p) | rnrm(p-1) | an,E(p-2) | ea,H,eh,NZ,WZ(p-3)) so each
  engine's in-order stream only carries data-ready work; engine balance:
  PE matmuls, ACT Prelu/Ln/Exp/E, DVE reduces + an(g0,g1) + ea/eh (PSUM
  readers), Pool sq + an(g2,g3).  A zero-matmul warmup ramps PE to full
  clock during the initial DMAs.  Activation tables are pinned to the one
  set containing Prelu/Ln/Exp so no per-pack table reloads occur.
  The device ships the raw NZ/WZ accumulators; the ~1 MFLOP epilogue
  (sim, the LSE over valid words, log()/lambda, margin loss) runs on
  host so the device tail is just two PSUM evictions + 2 DMAs.
"""

import sys

if "/opt/trn_rl_repo" not in sys.path:
    sys.path.insert(0, "/opt/trn_rl_repo")

import numpy as np

B, R, W, D = 64, 36, 50, 1024
NCORES = 8
ISH = 2                    # image shards
CSH = NCORES // ISH        # caption shards = 4
BI = B // ISH              # images per core = 32
CPC = B // CSH             # captions per core = 16
PACK = 3                   # images per pack
NPACK = 11                 # ceil(32 / 3) -> 33 rows incl. 1 pad image
BP = NPACK * PACK          # 33
PPART = PACK * R           # 108 partitions per pack
KCH = D // 128             # 8 contraction chunks
NGRP = 4                   # caption slot groups (4 slots each)
SPG = CPC // NGRP          # slots per group = 4
MIN_NCW = 256              # fp32r matmul needs moving free size >= 256
MAX_NCW = 512              # PSUM tiles must stay within one 2 KiB bank

MARGIN = 0.2
LAM_SM = 9.0
LAM_LSE = 6.0
EPS = 1e-8

_PROGRAM_CACHE: dict = {}


def _pin_act_tables():
    """Pin activation-function table selection to natural_log_exp_and_others
    (contains Prelu/Square/Ln/Exp/Copy) so the table-load insertion pass never
    ping-pongs between the exp-only and ln-only sets.  Returns a restore fn."""
    import concourse.bacc as bacc
    import concourse.hw_specs as hw_specs

    orig_hw, orig_bacc = hw_specs.get_activation_tables, bacc.get_activation_tables

    def pinned(arch):
        tabs = dict(orig_hw(arch))
        return {
            k: (v if k == "natural_log_exp_and_others" else frozenset())
            for k, v in tabs.items()
        }

    hw_specs.get_activation_tables = pinned
    bacc.get_activation_tables = pinned

    def restore():
        hw_specs.get_activation_tables = orig_hw
        bacc.get_activation_tables = orig_bacc

    return restore


def build_program(widths: tuple, debug: bool = False):
    """Build the per-core Bass/Tile program (identical on all 8 cores).
    widths[g]: padded word capacity of caption slots 4g..4g+3."""
    import concourse.bacc as bacc
    import concourse.mybir as mybir
    import concourse.tile as tile

    f32 = mybir.dt.float32
    f32r = mybir.dt.float32r
    bf16 = mybir.dt.bfloat16
    AF = mybir.ActivationFunctionType
    ALU = mybir.AluOpType
    AX = mybir.AxisListType

    assert len(widths) == NGRP
    goff = [SPG * sum(widths[:g]) for g in range(NGRP + 1)]  # column offsets
    NCW = goff[-1]
    assert MIN_NCW <= NCW <= MAX_NCW

    restore_tables = _pin_act_tables()
    nc = bacc.Bacc("TRN2", target_bir_lowering=False, debug=debug)

    # const AP for the Ln(s2 + 1e-30) bias (only 0.0/1.0 are pre-registered)
    _c30 = nc.alloc_sbuf_tensor("const-float32-1e-30", [128, 1], f32)
    nc.gpsimd.memset(_c30.ap(), 1e-30)
    nc.const_aps.aps[(f32, 1e-30)] = _c30.ap()
    nc.all_engine_barrier()

    im_d = nc.dram_tensor("im_packed", [NPACK, 128, KCH * PPART], bf16, kind="ExternalInput")
    s_d = nc.dram_tensor("s_packed", [128, KCH * NCW], bf16, kind="ExternalInput")
    g_d = nc.dram_tensor("gbd", [PPART, NPACK * PPART], f32, kind="ExternalInput")
    o_d = nc.dram_tensor("ones_w", [PPART, 2 * BP], f32, kind="ExternalInput")
    nz_d = nc.dram_tensor("nz_out", [BP, NCW], f32, kind="ExternalOutput")
    wz_d = nc.dram_tensor("wz_out", [BP, NCW], f32, kind="ExternalOutput")

    # Four-level software pipeline.  Iteration p issues:
    #   A-matmul(p+1) | Prelu(p) | rnrm(p-1) | an+E(p-2) | ea/H/eh/NZ/WZ(p-3)
    # so every op only consumes results from previous iterations and each
    # engine's in-order stream never waits on the current pack's chain.
    # PSUM: psA holds a_ps(p-3..p+1) = 5 banks, psH 1 (h is consumed by eh
    # within the same stage), accumulators 2 -> 8 banks total.
    LOOKAHEAD = 1
    EPI_DELAY = 3

    def grp(ap, g):
        """[PPART, NCW] access -> group-g 3D view [PPART, SPG, wg]."""
        return ap[:, goff[g]:goff[g + 1]].rearrange("p (c w) -> p c w", c=SPG)

    with tile.TileContext(nc) as tc:
        with (
            tc.tile_pool(name="const", bufs=1) as cpool,
            tc.tile_pool(name="imp", bufs=6) as impool,
            tc.tile_pool(name="work", bufs=8) as work,
            tc.tile_pool(name="small", bufs=8) as small,
            tc.tile_pool(name="ph2", bufs=1) as ph2,
            tc.tile_pool(name="psA", bufs=LOOKAHEAD + EPI_DELAY + 1, space="PSUM") as psA,
            tc.tile_pool(name="psH", bufs=1, space="PSUM") as psH,
            tc.tile_pool(name="psacc", bufs=1, space="PSUM") as psacc,
        ):
            s_sb = cpool.tile([128, KCH * NCW], bf16)
            g_all = cpool.tile([PPART, NPACK * PPART], f32)
            o_all = cpool.tile([PPART, 2 * BP], f32)

            def load_s():
                # 4 chunk DMAs so pack 0's first A-matmuls unblock early
                for c0, c1 in ((0, 1), (1, 3), (3, 6), (6, KCH)):
                    nc.sync.dma_start(
                        s_sb[:, c0 * NCW:c1 * NCW],
                        s_d[:, c0 * NCW:c1 * NCW],
                    )

            def load_go():
                # all Gram blocks in one wide tile (one DMA, full-rate rows);
                # pack p's stationary is a column slice; first use is
                # stage_b(0), EPI_DELAY iterations in.  ones: sliding window,
                # 3 block-ones columns at 63..65 so the [*, 63-3p : 129-3p]
                # slice has them at local columns 3p..3p+2.
                nc.sync.dma_start(g_all[:].bitcast(f32r), g_d[:].bitcast(f32r))
                nc.sync.dma_start(o_all[:].bitcast(f32r), o_d[:].bitcast(f32r))

            # persistent PSUM accumulators for the r-reductions
            nz_acc = psacc.tile([BP, NCW], f32)
            wz_acc = psacc.tile([BP, NCW], f32)

            im_tiles: dict = {}
            a_tiles: dict = {}

            def fetch(p, split=False):
                im_sb = impool.tile([128, KCH * PPART], bf16, tag="im")
                if split:
                    # k-chunk 0 lands first so A(p) k=0 can start immediately
                    nc.sync.dma_start(im_sb[:, 0:PPART], im_d[p, :, 0:PPART])
                    nc.sync.dma_start(im_sb[:, PPART:], im_d[p, :, PPART:])
                else:
                    nc.sync.dma_start(im_sb[:], im_d[p])
                im_tiles[p] = im_sb

            def a_matmul(p):
                # A[108, NCW] = sum_k im_pack_k^T @ s_k  (fp32r)
                im_sb = im_tiles.pop(p)
                a_ps = psA.tile([PPART, NCW], f32)
                for k in range(KCH):
                    nc.tensor.matmul(
                        a_ps[:],
                        im_sb[:, k * PPART:(k + 1) * PPART],
                        s_sb[:, k * NCW:(k + 1) * NCW],
                        start=(k == 0),
                        stop=(k == KCH - 1),
                    )
                a_tiles[p] = a_ps

            e_tiles: dict = {}
            al_tiles: dict = {}
            s2_tiles: dict = {}
            rn_tiles: dict = {}

            def leaky(p):
                """Al(p) = leaky_relu(A, 0.1) on ACT (Prelu)."""
                al = work.tile([PPART, NCW], f32, tag="al")
                nc.scalar.activation(al[:], a_tiles[p][:], AF.Prelu, alpha=0.1)
                al_tiles[p] = al

            def sq_s2(p):
                """s2(p)[108, 8] = sum_w Al^2 (square + group reduces on DVE;
                keeping sq off Pool breaks the sq->...->an->sq stream cycle)."""
                al = al_tiles[p]
                sq = work.tile([PPART, NCW], f32, tag="sq")
                nc.gpsimd.tensor_mul(sq[:], al[:], al[:])
                s2 = small.tile([PPART, CPC], f32, tag="s2")
                for g in range(NGRP):
                    nc.vector.tensor_reduce(
                        s2[:, SPG * g:SPG * (g + 1)], grp(sq, g), AX.X, ALU.add
                    )
                s2_tiles[p] = s2

            def rnrm_stage(p):
                """rnrm(p) = rsqrt(s2 + 1e-30) = exp(-0.5*ln(s2 + 1e-30));
                matches the reference 1/(sqrt(s2)+1e-8) to ~1e-10 rel."""
                s2 = s2_tiles.pop(p)
                lns = small.tile([PPART, CPC], f32, tag="lns")
                nc.scalar.activation(lns[:], s2[:], AF.Ln, bias=1e-30)
                rnrm = small.tile([PPART, CPC], f32, tag="rnrm")
                nc.scalar.activation(rnrm[:], lns[:], AF.Exp, scale=-0.5)
                rn_tiles[p] = rnrm

            def an_e(p):
                """An(p) = Al * rnrm (group-broadcast muls on Pool);
                E(p) = exp(9*An) on ACT."""
                al = al_tiles.pop(p)
                rnrm = rn_tiles.pop(p)
                an = work.tile([PPART, NCW], f32, tag="an")
                for g in range(NGRP):
                    eng = nc.vector if g < 2 else nc.gpsimd
                    eng.tensor_mul(
                        grp(an, g), grp(al, g),
                        rnrm[:, SPG * g:SPG * (g + 1)]
                        .broadcast_to([PPART, SPG, widths[g]]),
                    )
                e = work.tile([PPART, NCW], f32, tag="e")
                nc.scalar.activation(e[:].bitcast(f32r), an[:], AF.Exp, scale=LAM_SM)
                e_tiles[p] = e

            def stage_b(p):
                """EA/H/EH and the NZ/WZ accumulator matmuls for pack p."""
                first, last = (p == 0), (p == NPACK - 1)
                a_ps = a_tiles.pop(p)
                g_sb = g_all[:, PPART * p:PPART * (p + 1)]
                o_sb = o_all[:, BP - PACK - PACK * p:2 * BP - PACK - PACK * p]
                e = e_tiles.pop(p)

                # EA = E * A  (DVE; reads A from PSUM)
                ea = work.tile([PPART, NCW], f32, tag="ea")
                nc.vector.tensor_mul(ea[:].bitcast(f32r), e[:], a_ps[:])

                # H = Gbd @ E ; NZ += ones_p^T @ EA
                h_ps = psH.tile([PPART, NCW], f32, name="h_ps")
                nc.tensor.matmul(
                    h_ps[:], g_sb.bitcast(f32r), e[:].bitcast(f32r),
                    start=True, stop=True,
                )
                nc.tensor.matmul(
                    nz_acc[:], o_sb.bitcast(f32r), ea[:].bitcast(f32r),
                    start=first, stop=last,
                )

                # EH = E * H ; WZ += ones_p^T @ EH
                eh = work.tile([PPART, NCW], f32, tag="eh")
                nc.vector.tensor_mul(eh[:].bitcast(f32r), e[:], h_ps[:])
                nc.tensor.matmul(
                    wz_acc[:], o_sb.bitcast(f32r), eh[:].bitcast(f32r),
                    start=first, stop=last,
                )

            fetch(0, split=True)
            load_s()
            # PE p-state warmup: ~10 zero matmuls keep PE continuously busy
            # through the initial DMAs so A(0) runs at full clock (the ramp
            # needs ~3us of uninterrupted execution).
            warm = cpool.tile([128, 512], f32)
            nc.gpsimd.memset(warm[:], 0.0)
            for _ in range(10):
                h_ps = psH.tile([PPART, NCW], f32, name="h_ps")
                nc.tensor.matmul(
                    h_ps[:], warm[:, 0:PPART].bitcast(f32r),
                    warm[:, 0:NCW].bitcast(f32r),
                    start=True, stop=True,
                )
            for p in range(LOOKAHEAD):
                a_matmul(p)
                if p + 1 < LOOKAHEAD:
                    fetch(p + 1)
            load_go()

            # drained pipeline: iterate p over [0, NPACK+EPI_DELAY) issuing
            # each level for the pack it applies to (guards handle the edges).
            # Order matters per engine stream: stage_b first (ea/eh/H are
            # data-ready at iteration start), sq(p-1) is Pool's stream head
            # (so the sq->reduces->rnrm->an data path never cycles through
    	    # Pool's in-order stream), an/E afterwards.
            for p in range(NPACK + EPI_DELAY):
                if p + LOOKAHEAD < NPACK:
                    fetch(p + LOOKAHEAD)
                    a_matmul(p + LOOKAHEAD)
                if p < NPACK:
                    leaky(p)
                if 0 <= p - 1 < NPACK:
                    sq_s2(p - 1)
                    rnrm_stage(p - 1)
                if 0 <= p - 2 < NPACK:
                    an_e(p - 2)
                if 0 <= p - 3 < NPACK:
                    stage_b(p - 3)

            # ---- epilogue runs on host: ship the NZ / WZ accumulators.
            # (sim = NZ / max(cn*sqrt(WZ), eps*Z) with Z cancelled; the
            # LSE over valid words, log()/lambda and margin loss are a
            # ~1 MFLOP numpy epilogue, so the device tail is two PSUM
            # evictions on parallel engines + 2 DMAs.)
            nz_sb = ph2.tile([BP, NCW], f32)
            nc.scalar.activation(nz_sb[:], nz_acc[:], AF.Copy)
            wz_sb = ph2.tile([BP, NCW], f32)
            nc.vector.tensor_copy(wz_sb[:], wz_acc[:])
            nc.sync.dma_start(nz_d[:], nz_sb[:])
            nc.sync.dma_start(wz_d[:], wz_sb[:])

    nc.compile()
    restore_tables()
    return nc


def plan_packing(s_l: np.ndarray):
    """Sort captions by length, deal round-robin to (caption-col, slot), and
    pick the NGRP group widths.  Returns (perm[CSH, CPC] caption ids, widths).
    Slot j holds ranks CSH*j..CSH*j+3; group g spans slots 4g..4g+3."""
    s_l = np.asarray(s_l).astype(np.int64)
    order = np.argsort(-s_l, kind="stable")          # global rank -> caption
    perm = np.empty((CSH, CPC), np.int64)
    for r, cap in enumerate(order):
        perm[r % CSH, r // CSH] = cap
    lens = s_l[order]
    widths = [
        int(lens[CSH * SPG * g:CSH * SPG * (g + 1)].max()) for g in range(NGRP)
    ]
    # keep NCW >= MIN_NCW for full-rate fp32r matmuls (bump the last group)
    short = MIN_NCW - SPG * sum(widths)
    if short > 0:
        widths[-1] = min(W, widths[-1] + -(-short // SPG))
    return perm, tuple(widths)


def prepare_inputs(im: np.ndarray, s: np.ndarray, s_l: np.ndarray):
    """Host-side input marshalling: length-balanced ragged caption packing,
    d-major transposes, 3-image/108-partition im packs, block-diagonal Gram
    stationaries, caption norms."""
    import ml_dtypes

    bf16 = ml_dtypes.bfloat16
    im = np.ascontiguousarray(np.asarray(im, np.float32))
    s = np.ascontiguousarray(np.asarray(s, np.float32))
    s_l = np.asarray(s_l).astype(np.int64)

    perm, gw = plan_packing(s_l)
    NCW = SPG * sum(gw)
    widths = [gw[j // SPG] for j in range(CPC)]      # per-slot capacity
    offs = np.concatenate([[0], np.cumsum(widths)])[:-1]

    # zero out padded words so A columns for padded (c, w) are exactly 0
    wmask = (np.arange(W)[None, :] < s_l[:, None])          # [64, 50]
    s_z = s * wmask[:, :, None].astype(np.float32)
    cn = np.sqrt((s_z * s_z).sum(axis=2))                    # [64, 50]

    # per image-shard im packs [11, 128, 8*108] and Gram stationaries
    G = np.matmul(im, im.transpose(0, 2, 1))                 # [64, 36, 36] f32
    im_packed_s, gbd_s = [], []
    for i in range(ISH):
        imi = im[BI * i:BI * (i + 1)]
        imf = imi.transpose(2, 0, 1).reshape(D, BI * R)      # [1024, 1152]
        imf33 = np.zeros((D, BP * R), np.float32)
        imf33[:, : BI * R] = imf
        im_packed_s.append(np.ascontiguousarray(
            imf33.reshape(KCH, 128, NPACK, PPART)
            .transpose(2, 1, 0, 3)
            .reshape(NPACK, 128, KCH * PPART)
            .astype(bf16)
        ))
        gbd = np.zeros((PPART, NPACK * PPART), np.float32)
        for j in range(PACK):
            for p in range(NPACK):
                b = BI * i + PACK * p + j
                if PACK * p + j < BI:
                    gbd[R * j : R * (j + 1),
                        PPART * p + R * j : PPART * p + R * (j + 1)] = G[b]
        gbd_s.append(gbd)

    # sliding-window ones [108, 66]: 3 block-ones columns at BP-3..BP-1; pack
    # p's stationary is the [*, BP-3-3p : 2*BP-3-3p] slice
    ones_w = np.zeros((PPART, 2 * BP), np.float32)
    for j in range(PACK):
        ones_w[R * j : R * (j + 1), BP - PACK + j] = 1.0

    # per caption-col packed s; shared across the 2 image shards
    s_packed_c = []
    for c in range(CSH):
        caps = perm[c]                                        # 16 caption ids
        sT = np.zeros((D, NCW), np.float32)
        for j, cap in enumerate(caps):
            L = int(s_l[cap])
            sT[:, offs[j]:offs[j] + L] = s_z[cap, :L].T
        s_packed_c.append(np.ascontiguousarray(
            sT.reshape(KCH, 128, NCW).transpose(1, 0, 2)
            .reshape(128, KCH * NCW).astype(bf16)
        ))

    in_maps = []
    for i in range(ISH):
        for c in range(CSH):
            in_maps.append(
                {
                    "im_packed": im_packed_s[i],
                    "s_packed": s_packed_c[c],
                    "gbd": gbd_s[i],
                    "ones_w": ones_w,
                }
            )
    return in_maps, (cn, offs, widths), perm, gw


def margin_loss(scores: np.ndarray) -> np.float32:
    scores = scores.astype(np.float32)
    diag = np.diag(scores).copy()
    cost_s = np.maximum(MARGIN + scores - diag[:, None], 0.0)
    cost_im = np.maximum(MARGIN + scores - diag[None, :], 0.0)
    np.fill_diagonal(cost_s, 0.0)
    np.fill_diagonal(cost_im, 0.0)
    return np.float32(cost_s.max(axis=1).sum() + cost_im.max(axis=0).sum())


def _kernel_numpy(im, s, s_l):
    """Correctness fallback (exercised only for caption-length profiles whose
    packed width exceeds the PSUM single-bank budget; never for the reference
    inputs).  Mirrors the reference math in numpy."""
    im = np.asarray(im, np.float64)
    s = np.asarray(s, np.float64)
    s_l = np.asarray(s_l).astype(np.int64)
    A = np.einsum("brd,cwd->cbrw", im, s)
    mask = np.arange(s.shape[1])[None, :] < s_l[:, None]
    Al = np.where(A > 0, A, 0.1 * A) * mask[:, None, None, :]
    nrm = np.sqrt((Al * Al).sum(axis=3, keepdims=True)) + EPS
    An = Al / nrm
    Sm = np.exp(An * LAM_SM)
    Sm /= Sm.sum(axis=2, keepdims=True)
    num = (Sm * A).sum(axis=2)
    Gm = np.einsum("brd,bsd->brs", im, im)
    wsq = np.einsum("cbrw,brs,cbsw->cbw", Sm, Gm, Sm)
    wn = np.sqrt(np.maximum(wsq, 0.0))
    cn = np.sqrt((s * s).sum(axis=2))
    sim = num / np.maximum(cn[:, None, :] * wn, EPS)
    e = np.where(mask[:, None, :], np.exp(sim * LAM_LSE), 0.0)
    scores = (np.log(e.sum(axis=2)) / LAM_LSE).T
    return margin_loss(scores.astype(np.float32))


def kernel(im: np.ndarray, s: np.ndarray, s_l: np.ndarray) -> np.ndarray:
    from concourse.bass_utils import run_bass_kernel_spmd

    _, gw = plan_packing(s_l)
    if SPG * sum(gw) > MAX_NCW:
        return _kernel_numpy(im, s, s_l)

    in_maps, (cn, offs, widths), perm, key = prepare_inputs(im, s, s_l)
    if key not in _PROGRAM_CACHE:
        _PROGRAM_CACHE[key] = build_program(key)
    nc = _PROGRAM_CACHE[key]

    s_l = np.asarray(s_l).astype(np.int64)
    res = run_bass_kernel_spmd(nc, in_maps, list(range(NCORES))).results
    scores = np.empty((B, B), np.float32)
    for i in range(ISH):
        for c in range(CSH):
            r = res[CSH * i + c]
            nz = r["nz_out"][:BI].astype(np.float64)          # [32, NCW]
            wz = r["wz_out"][:BI].astype(np.float64)
            for j, cap in enumerate(perm[c]):
                L = int(s_l[cap])
                sl = slice(int(offs[j]), int(offs[j]) + L)
                den = np.maximum(
                    cn[cap, :L][None, :] * np.sqrt(np.maximum(wz[:, sl], 0.0)),
                    EPS,
                )
                sim = nz[:, sl] / den
                scores[BI * i:BI * (i + 1), cap] = (
                    np.log(np.exp(sim * LAM_LSE).sum(axis=1)) / LAM_LSE
                )
    return margin_loss(scores)


# revision 74
# speedup vs baseline: 1.0072x; 1.0072x over previous
"""Trainium2 Bass kernel for nn_ContrastiveLoss (ragged_sequence).

Math (see reference): a cross-attention t2i score matrix scores[i, c] over
B=64 images x B=64 captions, then a max-violation margin loss.

Sharding (2-way images x 4-way captions over 8 NeuronCores): each core
holds 32 images and 16 captions and computes its [32, 16] block of the
score matrix; the tiny 64x64 margin-loss reduction runs on host.

Ragged packing: captions are sorted by length and dealt round-robin to
(caption-col, slot) so slot j holds the 4 captions of rank 4j..4j+3.
Slots are padded to four uniform group widths (4 slots per group), so
per-caption word reductions are 4 static strided views instead of 16
ragged ones.  NCW = 4*sum(widths) (~496 for the reference inputs vs 800
dense) and is kept in [256, 512]: >= 256 so fp32r matmuls stream at
1 row/cycle, <= 512 so every [*, NCW] f32 PSUM tile fits one 2 KiB bank.

Device layout (per core):
  Images are processed in 11 "packs" of 3 (32 real + 1 zero-pad).  A pack
  occupies 108 partitions = 3 images x 36 regions.  The main matmul
  A = im @ s^T uses stationary = im-pack [128d, 108br] and moving =
  s^T [128d, NCW], both bf16 (PSUM accumulates f32; the downstream
  normalization cancels most of the rounding), 8 K-chunks of d.
  Word-axis (w) reductions are free-axis DVE reduces over the group
  views; region-axis (r) reductions are PE matmuls:
    - H = Gbd @ E with Gbd = blockdiag(G[b0],G[b1],G[b2]) Gram blocks,
      all packs' stationaries column-concatenated in one SBUF tile
    - NZ/WZ accumulate ones_p^T @ {E*A, E*H} over packs in PSUM, where
      ones_p is a 66-column slice of one sliding-window [108, 132] tile.
  The softmax normalizer Z cancels in sim = (NZ/Z)/(cn*sqrt(WZ)/Z), so it
  is never computed.  The per-pack chain is software-pipelined four deep
  (A(p+1) | Prelu(p) | rnrm(p-1) | an,E(p-2) | ea,H,eh,NZ,WZ(p-3)) so each
  engine's in-order stream only carries data-ready work; engine balance:
  PE matmuls, ACT Prelu/Ln/Exp/E, DVE reduces + an(g0,g1) + ea/eh (PSUM
  readers), Pool sq + an(g2,g3).  A zero-matmul warmup ramps PE to full
  clock during the initial DMAs.  Activation tables are pinned to the one
  set containing Prelu/Ln/Exp so no per-pack table reloads occur.
  The device ships the raw NZ/WZ accumulators; the ~1 MFLOP epilogue
  (sim, the LSE over valid words, log()/lambda, margin loss) runs on
  host so the device tail is just two PSUM evictions + 2 DMAs.
"""

import sys

if "/opt/trn_rl_repo" not in sys.path:
    sys.path.insert(0, "/opt/trn_rl_repo")

import numpy as np

B, R, W, D = 64, 36, 50, 1024
NCORES = 8
ISH = 2                    # image shards
CSH = NCORES // ISH        # caption shards = 4
BI = B // ISH              # images per core = 32
CPC = B // CSH             # captions per core = 16
PACK = 3                   # images per pack
NPACK = 11                 # ceil(32 / 3) -> 33 rows incl. 1 pad image
BP = NPACK * PACK          # 33
PPART = PACK * R           # 108 partitions per pack
KCH = D // 128             # 8 contraction chunks
NGRP = 4                   # caption slot groups (4 slots each)
SPG = CPC // NGRP          # slots per group = 4
MIN_NCW = 256              # fp32r matmul needs moving free size >= 256
MAX_NCW = 512              # PSUM tiles must stay within one 2 KiB bank

MARGIN = 0.2
LAM_SM = 9.0
LAM_LSE = 6.0
EPS = 1e-8

_PROGRAM_CACHE: dict = {}


def _pin_act_tables():
    """Pin activation-function table selection to natural_log_exp_and_others
    (contains Prelu/Square/Ln/Exp/Copy) so the table-load insertion pass never
    ping-pongs between the exp-only and ln-only sets.  Returns a restore fn."""
    import concourse.bacc as bacc
    import concourse.hw_specs as hw_specs

    orig_hw, orig_bacc = hw_specs.get_activation_tables, bacc.get_activation_tables

    def pinned(arch):
        tabs = dict(orig_hw(arch))
        return {
            k: (v if k == "natural_log_exp_and_others" else frozenset())
            for k, v in tabs.items()
        }

    hw_specs.get_activation_tables = pinned
    bacc.get_activation_tables = pinned

    def restore():
        hw_specs.get_activation_tables = orig_hw
        bacc.get_activation_tables = orig_bacc

    return restore


def build_program(widths: tuple, debug: bool = False):
    """Build the per-core Bass/Tile program (identical on all 8 cores).
    widths[g]: padded word capacity of caption slots 4g..4g+3."""
    import concourse.bacc as bacc
    import concourse.mybir as mybir
    import concourse.tile as tile

    f32 = mybir.dt.float32
    f32r = mybir.dt.float32r
    bf16 = mybir.dt.bfloat16
    AF = mybir.ActivationFunctionType
    ALU = mybir.AluOpType
    AX = mybir.AxisListType

    assert len(widths) == NGRP
    goff = [SPG * sum(widths[:g]) for g in range(NGRP + 1)]  # column offsets
    NCW = goff[-1]
    assert MIN_NCW <= NCW <= MAX_NCW

    restore_tables = _pin_act_tables()
    nc = bacc.Bacc("TRN2", target_bir_lowering=False, debug=debug)

    # const AP for the Ln(s2 + 1e-30) bias (only 0.0/1.0 are pre-registered)
    _c30 = nc.alloc_sbuf_tensor("const-float32-1e-30", [128, 1], f32)
    nc.gpsimd.memset(_c30.ap(), 1e-30)
    nc.const_aps.aps[(f32, 1e-30)] = _c30.ap()
    nc.all_engine_barrier()

    im_d = nc.dram_tensor("im_packed", [NPACK, 128, KCH * PPART], bf16, kind="ExternalInput")
    s_d = nc.dram_tensor("s_packed", [128, KCH * NCW], bf16, kind="ExternalInput")
    g_d = nc.dram_tensor("gbd", [PPART, NPACK * PPART], f32, kind="ExternalInput")
    o_d = nc.dram_tensor("ones_w", [PPART, 2 * BP], f32, kind="ExternalInput")
    nz_d = nc.dram_tensor("nz_out", [BP, NCW], f32, kind="ExternalOutput")
    wz_d = nc.dram_tensor("wz_out", [BP, NCW], f32, kind="ExternalOutput")

    # Four-level software pipeline.  Iteration p issues:
    #   A-matmul(p+1) | Prelu(p) | rnrm(p-1) | an+E(p-2) | ea/H/eh/NZ/WZ(p-3)
    # so every op only consumes results from previous iterations and each
    # engine's in-order stream never waits on the current pack's chain.
    # PSUM: psA holds a_ps(p-3..p+1) = 5 banks, psH 1 (h is consumed by eh
    # within the same stage), accumulators 2 -> 8 banks total.
    LOOKAHEAD = 1
    EPI_DELAY = 3

    def grp(ap, g):
        """[PPART, NCW] access -> group-g 3D view [PPART, SPG, wg]."""
        return ap[:, goff[g]:goff[g + 1]].rearrange("p (c w) -> p c w", c=SPG)

    with tile.TileContext(nc) as tc:
        with (
            tc.tile_pool(name="const", bufs=1) as cpool,
            tc.tile_pool(name="imp", bufs=6) as impool,
            tc.tile_pool(name="work", bufs=8) as work,
            tc.tile_pool(name="small", bufs=8) as small,
            tc.tile_pool(name="ph2", bufs=1) as ph2,
            tc.tile_pool(name="psA", bufs=LOOKAHEAD + EPI_DELAY + 1, space="PSUM") as psA,
            tc.tile_pool(name="psH", bufs=1, space="PSUM") as psH,
            tc.tile_pool(name="psacc", bufs=1, space="PSUM") as psacc,
        ):
            s_sb = cpool.tile([128, KCH * NCW], bf16)
            g_all = cpool.tile([PPART, NPACK * PPART], f32)
            o_all = cpool.tile([PPART, 2 * BP], f32)

            def load_s():
                # 4 chunk DMAs so pack 0's first A-matmuls unblock early
                for c0, c1 in ((0, 1), (1, 3), (3, 6), (6, KCH)):
                    nc.sync.dma_start(
                        s_sb[:, c0 * NCW:c1 * NCW],
                        s_d[:, c0 * NCW:c1 * NCW],
                    )

            def load_go():
                # all Gram blocks in one wide tile (one DMA, full-rate rows);
                # pack p's stationary is a column slice; first use is
                # stage_b(0), EPI_DELAY iterations in.  ones: sliding window,
                # 3 block-ones columns at 63..65 so the [*, 63-3p : 129-3p]
                # slice has them at local columns 3p..3p+2.
                nc.sync.dma_start(g_all[:].bitcast(f32r), g_d[:].bitcast(f32r))
                nc.sync.dma_start(o_all[:].bitcast(f32r), o_d[:].bitcast(f32r))

            # persistent PSUM accumulators for the r-reductions
            nz_acc = psacc.tile([BP, NCW], f32)
            wz_acc = psacc.tile([BP, NCW], f32)

            im_tiles: dict = {}
            a_tiles: dict = {}

            def fetch(p, split=False):
                im_sb = impool.tile([128, KCH * PPART], bf16, tag="im")
                if split:
                    # k-chunk 0 lands first so A(p) k=0 can start immediately
                    nc.sync.dma_start(im_sb[:, 0:PPART], im_d[p, :, 0:PPART])
                    nc.sync.dma_start(im_sb[:, PPART:], im_d[p, :, PPART:])
                else:
                    nc.sync.dma_start(im_sb[:], im_d[p])
                im_tiles[p] = im_sb

            def a_matmul(p):
                # A[108, NCW] = sum_k im_pack_k^T @ s_k  (fp32r)
                im_sb = im_tiles.pop(p)
                a_ps = psA.tile([PPART, NCW], f32)
                for k in range(KCH):
                    nc.tensor.matmul(
                        a_ps[:],
                        im_sb[:, k * PPART:(k + 1) * PPART],
                        s_sb[:, k * NCW:(k + 1) * NCW],
                        start=(k == 0),
                        stop=(k == KCH - 1),
                    )
                a_tiles[p] = a_ps

            e_tiles: dict = {}
            al_tiles: dict = {}
            s2_tiles: dict = {}
            rn_tiles: dict = {}

            def leaky(p):
                """Al(p) = leaky_relu(A, 0.1) on ACT (Prelu)."""
                al = work.tile([PPART, NCW], f32, tag="al")
                nc.scalar.activation(al[:], a_tiles[p][:], AF.Prelu, alpha=0.1)
                al_tiles[p] = al

            def sq_s2(p):
                """s2(p)[108, 8] = sum_w Al^2 (square + group reduces on DVE;
                keeping sq off Pool breaks the sq->...->an->sq stream cycle)."""
                al = al_tiles[p]
                sq = work.tile([PPART, NCW], f32, tag="sq")
                nc.gpsimd.tensor_mul(sq[:], al[:], al[:])
                s2 = small.tile([PPART, CPC], f32, tag="s2")
                for g in range(NGRP):
                    nc.vector.tensor_reduce(
                        s2[:, SPG * g:SPG * (g + 1)], grp(sq, g), AX.X, ALU.add
                    )
                s2_tiles[p] = s2

            def rnrm_stage(p):
                """rnrm(p) = rsqrt(s2 + 1e-30) = exp(-0.5*ln(s2 + 1e-30));
                matches the reference 1/(sqrt(s2)+1e-8) to ~1e-10 rel."""
                s2 = s2_tiles.pop(p)
                lns = small.tile([PPART, CPC], f32, tag="lns")
                nc.scalar.activation(lns[:], s2[:], AF.Ln, bias=1e-30)
                rnrm = small.tile([PPART, CPC], f32, tag="rnrm")
                nc.scalar.activation(rnrm[:], lns[:], AF.Exp, scale=-0.5)
                rn_tiles[p] = rnrm

            def an_e(p):
                """An(p) = Al * rnrm (group-broadcast muls on Pool);
                E(p) = exp(9*An) on ACT."""
                al = al_tiles.pop(p)
                rnrm = rn_tiles.pop(p)
                an = work.tile([PPART, NCW], f32, tag="an")
                for g in range(NGRP):
                    eng = nc.vector if g < 2 else nc.gpsimd
                    eng.tensor_mul(
                        grp(an, g), grp(al, g),
                        rnrm[:, SPG * g:SPG * (g + 1)]
                        .broadcast_to([PPART, SPG, widths[g]]),
                    )
                e = work.tile([PPART, NCW], f32, tag="e")
                nc.scalar.activation(e[:].bitcast(f32r), an[:], AF.Exp, scale=LAM_SM)
                e_tiles[p] = e

            def stage_b(p):
                """EA/H/EH and the NZ/WZ accumulator matmuls for pack p."""
                first, last = (p == 0), (p == NPACK - 1)
                a_ps = a_tiles.pop(p)
                g_sb = g_all[:, PPART * p:PPART * (p + 1)]
                o_sb = o_all[:, BP - PACK - PACK * p:2 * BP - PACK - PACK * p]
                e = e_tiles.pop(p)

                # EA = E * A  (DVE; reads A from PSUM)
                ea = work.tile([PPART, NCW], f32, tag="ea")
                nc.vector.tensor_mul(ea[:].bitcast(f32r), e[:], a_ps[:])

                # H = Gbd @ E ; NZ += ones_p^T @ EA
                h_ps = psH.tile([PPART, NCW], f32, name="h_ps")
                nc.tensor.matmul(
                    h_ps[:], g_sb.bitcast(f32r), e[:].bitcast(f32r),
                    start=True, stop=True,
                )
                nc.tensor.matmul(
                    nz_acc[:], o_sb.bitcast(f32r), ea[:].bitcast(f32r),
                    start=first, stop=last,
                )

                # EH = E * H ; WZ += ones_p^T @ EH
                eh = work.tile([PPART, NCW], f32, tag="eh")
                nc.vector.tensor_mul(eh[:].bitcast(f32r), e[:], h_ps[:])
                nc.tensor.matmul(
                    wz_acc[:], o_sb.bitcast(f32r), eh[:].bitcast(f32r),
                    start=first, stop=last,
                )

            fetch(0, split=True)
            load_s()
            for _pf in range(1, min(4, NPACK)):
                fetch(_pf)
            # PE p-state warmup: ~10 zero matmuls keep PE continuously busy
            # through the initial DMAs so A(0) runs at full clock (the ramp
            # needs ~3us of uninterrupted execution).
            warm = cpool.tile([128, 512], f32)
            nc.gpsimd.memset(warm[:], 0.0)
            for _ in range(10):
                h_ps = psH.tile([PPART, NCW], f32, name="h_ps")
                nc.tensor.matmul(
                    h_ps[:], warm[:, 0:PPART].bitcast(f32r),
                    warm[:, 0:NCW].bitcast(f32r),
                    start=True, stop=True,
                )
            for p in range(LOOKAHEAD):
                a_matmul(p)
                if p + 1 < LOOKAHEAD:
                    fetch(p + 1)
            load_go()

            # drained pipeline: iterate p over [0, NPACK+EPI_DELAY) issuing
            # each level for the pack it applies to (guards handle the edges).
            # Order matters per engine stream: stage_b first (ea/eh/H are
            # data-ready at iteration start), sq(p-1) is Pool's stream head
            # (so the sq->reduces->rnrm->an data path never cycles through
    	    # Pool's in-order stream), an/E afterwards.
            for p in range(NPACK + EPI_DELAY):
                if p + 4 < NPACK:
                    fetch(p + 4)
                if p + LOOKAHEAD < NPACK:
                    a_matmul(p + LOOKAHEAD)
                if p < NPACK:
                    leaky(p)
                if 0 <= p - 1 < NPACK:
                    sq_s2(p - 1)
                    rnrm_stage(p - 1)
                if 0 <= p - 2 < NPACK:
                    an_e(p - 2)
                if 0 <= p - 3 < NPACK:
                    stage_b(p - 3)

            # ---- epilogue runs on host: ship the NZ / WZ accumulators.
            # (sim = NZ / max(cn*sqrt(WZ), eps*Z) with Z cancelled; the
            # LSE over valid words, log()/lambda and margin loss are a
            # ~1 MFLOP numpy epilogue, so the device tail is two PSUM
            # evictions on parallel engines + 2 DMAs.)
            nz_sb = ph2.tile([BP, NCW], f32)
            nc.scalar.activation(nz_sb[:], nz_acc[:], AF.Copy)
            wz_sb = ph2.tile([BP, NCW], f32)
            nc.vector.tensor_copy(wz_sb[:], wz_acc[:])
            nc.sync.dma_start(nz_d[:], nz_sb[:])
            nc.sync.dma_start(wz_d[:], wz_sb[:])

    nc.compile()
    restore_tables()
    return nc


def plan_packing(s_l: np.ndarray):
    """Sort captions by length, deal round-robin to (caption-col, slot), and
    pick the NGRP group widths.  Returns (perm[CSH, CPC] caption ids, widths).
    Slot j holds ranks CSH*j..CSH*j+3; group g spans slots 4g..4g+3."""
    s_l = np.asarray(s_l).astype(np.int64)
    order = np.argsort(-s_l, kind="stable")          # global rank -> caption
    perm = np.empty((CSH, CPC), np.int64)
    for r, cap in enumerate(order):
        perm[r % CSH, r // CSH] = cap
    lens = s_l[order]
    widths = [
        int(lens[CSH * SPG * g:CSH * SPG * (g + 1)].max()) for g in range(NGRP)
    ]
    # keep NCW >= MIN_NCW for full-rate fp32r matmuls (bump the last group)
    short = MIN_NCW - SPG * sum(widths)
    if short > 0:
        widths[-1] = min(W, widths[-1] + -(-short // SPG))
    return perm, tuple(widths)


def prepare_inputs(im: np.ndarray, s: np.ndarray, s_l: np.ndarray):
    """Host-side input marshalling: length-balanced ragged caption packing,
    d-major transposes, 3-image/108-partition im packs, block-diagonal Gram
    stationaries, caption norms."""
    import ml_dtypes

    bf16 = ml_dtypes.bfloat16
    im = np.ascontiguousarray(np.asarray(im, np.float32))
    s = np.ascontiguousarray(np.asarray(s, np.float32))
    s_l = np.asarray(s_l).astype(np.int64)

    perm, gw = plan_packing(s_l)
    NCW = SPG * sum(gw)
    widths = [gw[j // SPG] for j in range(CPC)]      # per-slot capacity
    offs = np.concatenate([[0], np.cumsum(widths)])[:-1]

    # zero out padded words so A columns for padded (c, w) are exactly 0
    wmask = (np.arange(W)[None, :] < s_l[:, None])          # [64, 50]
    s_z = s * wmask[:, :, None].astype(np.float32)
    cn = np.sqrt((s_z * s_z).sum(axis=2))                    # [64, 50]

    # per image-shard im packs [11, 128, 8*108] and Gram stationaries
    G = np.matmul(im, im.transpose(0, 2, 1))                 # [64, 36, 36] f32
    im_packed_s, gbd_s = [], []
    for i in range(ISH):
        imi = im[BI * i:BI * (i + 1)]
        imf = imi.transpose(2, 0, 1).reshape(D, BI * R)      # [1024, 1152]
        imf33 = np.zeros((D, BP * R), np.float32)
        imf33[:, : BI * R] = imf
        im_packed_s.append(np.ascontiguousarray(
            imf33.reshape(KCH, 128, NPACK, PPART)
            .transpose(2, 1, 0, 3)
            .reshape(NPACK, 128, KCH * PPART)
            .astype(bf16)
        ))
        gbd = np.zeros((PPART, NPACK * PPART), np.float32)
        for j in range(PACK):
            for p in range(NPACK):
                b = BI * i + PACK * p + j
                if PACK * p + j < BI:
                    gbd[R * j : R * (j + 1),
                        PPART * p + R * j : PPART * p + R * (j + 1)] = G[b]
        gbd_s.append(gbd)

    # sliding-window ones [108, 66]: 3 block-ones columns at BP-3..BP-1; pack
    # p's stationary is the [*, BP-3-3p : 2*BP-3-3p] slice
    ones_w = np.zeros((PPART, 2 * BP), np.float32)
    for j in range(PACK):
        ones_w[R * j : R * (j + 1), BP - PACK + j] = 1.0

    # per caption-col packed s; shared across the 2 image shards
    s_packed_c = []
    for c in range(CSH):
        caps = perm[c]                                        # 16 caption ids
        sT = np.zeros((D, NCW), np.float32)
        for j, cap in enumerate(caps):
            L = int(s_l[cap])
            sT[:, offs[j]:offs[j] + L] = s_z[cap, :L].T
        s_packed_c.append(np.ascontiguousarray(
            sT.reshape(KCH, 128, NCW).transpose(1, 0, 2)
            .reshape(128, KCH * NCW).astype(bf16)
        ))

    in_maps = []
    for i in range(ISH):
        for c in range(CSH):
            in_maps.append(
                {
                    "im_packed": im_packed_s[i],
                    "s_packed": s_packed_c[c],
                    "gbd": gbd_s[i],
                    "ones_w": ones_w,
                }
            )
    return in_maps, (cn, offs, widths), perm, gw


def margin_loss(scores: np.ndarray) -> np.float32:
    scores = scores.astype(np.float32)
    diag = np.diag(scores).copy()
    cost_s = np.maximum(MARGIN + scores - diag[:, None], 0.0)
    cost_im = np.maximum(MARGIN + scores - diag[None, :], 0.0)
    np.fill_diagonal(cost_s, 0.0)
    np.fill_diagonal(cost_im, 0.0)
    return np.float32(cost_s.max(axis=1).sum() + cost_im.max(axis=0).sum())


def _kernel_numpy(im, s, s_l):
    """Correctness fallback (exercised only for caption-length profiles whose
    packed width exceeds the PSUM single-bank budget; never for the reference
    inputs).  Mirrors the reference math in numpy."""
    im = np.asarray(im, np.float64)
    s = np.asarray(s, np.float64)
    s_l = np.asarray(s_l).astype(np.int64)
    A = np.einsum("brd,cwd->cbrw", im, s)
    mask = np.arange(s.shape[1])[None, :] < s_l[:, None]
    Al = np.where(A > 0, A, 0.1 * A) * mask[:, None, None, :]
    nrm = np.sqrt((Al * Al).sum(axis=3, keepdims=True)) + EPS
    An = Al / nrm
    Sm = np.exp(An * LAM_SM)
    Sm /= Sm.sum(axis=2, keepdims=True)
    num = (Sm * A).sum(axis=2)
    Gm = np.einsum("brd,bsd->brs", im, im)
    wsq = np.einsum("cbrw,brs,cbsw->cbw", Sm, Gm, Sm)
    wn = np.sqrt(np.maximum(wsq, 0.0))
    cn = np.sqrt((s * s).sum(axis=2))
    sim = num / np.maximum(cn[:, None, :] * wn, EPS)
    e = np.where(mask[:, None, :], np.exp(sim * LAM_LSE), 0.0)
    scores = (np.log(e.sum(axis=2)) / LAM_LSE).T
    return margin_loss(scores.astype(np.float32))


def kernel(im: np.ndarray, s: np.ndarray, s_l: np.ndarray) -> np.ndarray:
    from concourse.bass_utils import run_bass_kernel_spmd

    _, gw = plan_packing(s_l)
    if SPG * sum(gw) > MAX_NCW:
        return _kernel_numpy(im, s, s_l)

    in_maps, (cn, offs, widths), perm, key = prepare_inputs(im, s, s_l)
    if key not in _PROGRAM_CACHE:
        _PROGRAM_CACHE[key] = build_program(key)
    nc = _PROGRAM_CACHE[key]

    s_l = np.asarray(s_l).astype(np.int64)
    res = run_bass_kernel_spmd(nc, in_maps, list(range(NCORES))).results
    scores = np.empty((B, B), np.float32)
    for i in range(ISH):
        for c in range(CSH):
            r = res[CSH * i + c]
            nz = r["nz_out"][:BI].astype(np.float64)          # [32, NCW]
            wz = r["wz_out"][:BI].astype(np.float64)
            for j, cap in enumerate(perm[c]):
                L = int(s_l[cap])
                sl = slice(int(offs[j]), int(offs[j]) + L)
                den = np.maximum(
                    cn[cap, :L][None, :] * np.sqrt(np.maximum(wz[:, sl], 0.0)),
                    EPS,
                )
                sim = nz[:, sl] / den
                scores[BI * i:BI * (i + 1), cap] = (
                    np.log(np.exp(sim * LAM_LSE).sum(axis=1)) / LAM_LSE
                )
    return margin_loss(scores)
